# revision 4
# baseline (speedup 1.0000x reference)
"""Trainium2 Bass kernel for nn_CustomDense (bit-serial quantized dense layer).

Math: the reference's per-element bit-serial shift-add loop computes exactly
    f(x, w) = trunc(x * w / 256)          (bits=8, x in [0,15], w in [-128,127])
so  out = relu(sum_j f(x_ij, w_ju) + bias_u).

Device algorithm (exact, integer-precise):
  - one-hot over the 15 nonzero activation values v:
        sum_j f = sum_v (X==v) @ floor(v*W/256)  + trunc-floor correction
  - floor(v*W/256) is computed on-chip as round(v*W/256 - 511/1024) using the
    DVE's round-to-nearest fp32->int16 output converter (exact on the 1/256
    grid), then cast int16 -> bf16 for the PE.
  - trunc vs floor differs by 1 exactly when w<0 and x*|w| % 256 != 0; this
    collapses to 8 rank-1-mask matmul groups:
        + Xnz@Mneg - XP1@D128 - XP2@(D64+D128) - XP3@(D32+D64+D96+D128)
    with Xnz=1[x>=1], XPk=1[v2(x)=k], Dm=1[w=-m] (D-masks negated on-chip).

Sharding: D (contraction, 1024) split across 8 cores, 128 rows each; every
core computes a full [64, 1024] partial in PSUM (all values are small
integers, fp32-exact). Host sums the 8 partials (exact), adds bias in fp32
and applies relu -- bit-identical to the reference.
"""

import numpy as np

B, D, U, BITS = 64, 1024, 1024, 8
NCORES = 8
DSH = D // NCORES  # 128 contraction rows per core
OFF = -511.0 / 1024.0  # round(y + OFF) == floor(y) for y on the 1/256 grid

# TC engine assignment per v (int16 -> bf16 convert): "act" or "dve"
TC_ENGINE = {v: ("act" if v % 2 == 0 else "dve") for v in range(1, 16)}
TRACE = False

_NC_CACHE = {}


def _build_nc():
    import concourse.bacc as bacc
    import concourse.mybir as mybir
    import concourse.tile as tile

    Alu = mybir.AluOpType
    bf16 = mybir.dt.bfloat16
    i16 = mybir.dt.int16
    f32 = mybir.dt.float32

    nc = bacc.Bacc("TRN2", target_bir_lowering=False, debug=False)
    xt_d = nc.dram_tensor("xt", [DSH, B], i16, kind="ExternalInput")
    w_d = nc.dram_tensor("w", [DSH, U], i16, kind="ExternalInput")
    out_d = nc.dram_tensor("out", [B, U], f32, kind="ExternalOutput")

    with tile.TileContext(nc) as tc:
        with (
            tc.tile_pool(name="io", bufs=1) as io,
            tc.tile_pool(name="gi", bufs=4) as gi_pool,
            tc.tile_pool(name="gb", bufs=4) as gb_pool,
            tc.tile_pool(name="ps", bufs=1, space="PSUM") as ps,
        ):
            xt_sb = io.tile([DSH, B], i16)
            w_sb = io.tile([DSH, U], i16)
            nc.sync.dma_start(xt_sb[:], xt_d[:])
            nc.sync.dma_start(w_sb[:], w_d[:])

            # --- X-side masks (bf16 [DSH, B]) ---
            h = {}
            for v in range(1, 16):
                t = io.tile([DSH, B], bf16, tag=f"h{v}")
                nc.vector.tensor_scalar(
                    out=t[:], in0=xt_sb[:], scalar1=float(v), scalar2=None,
                    op0=Alu.is_equal,
                )
                h[v] = t
            xnz = io.tile([DSH, B], bf16, tag="xnz")
            nc.vector.tensor_scalar(
                out=xnz[:], in0=xt_sb[:], scalar1=1.0, scalar2=None,
                op0=Alu.is_ge,
            )
            # --- W-side correction masks (bf16 [DSH, U]); D-masks negated ---
            mneg = io.tile([DSH, U], bf16, tag="mneg")
            nc.vector.tensor_scalar(
                out=mneg[:], in0=w_sb[:], scalar1=0.0, scalar2=None,
                op0=Alu.is_lt,
            )
            dmask = {}
            for m in (128, 64, 32, 96):
                t = io.tile([DSH, U], bf16, tag=f"d{m}")
                nc.vector.tensor_scalar(
                    out=t[:], in0=w_sb[:], scalar1=float(-m), scalar2=-1.0,
                    op0=Alu.is_equal, op1=Alu.mult,
                )
                dmask[m] = t

            # --- G_v = floor(v*W/256): fp32 mult+add, round-to-int16 ---
            g_bf = {}
            for v in range(1, 16):
                g_i = gi_pool.tile([DSH, U], i16, tag=f"gi{v % 4}")
                nc.vector.tensor_scalar(
                    out=g_i[:], in0=w_sb[:], scalar1=float(v) / 256.0, scalar2=OFF,
                    op0=Alu.mult, op1=Alu.add,
                )
                t = gb_pool.tile([DSH, U], bf16, tag=f"gb{v}")
                if TC_ENGINE[v] == "act":
                    nc.scalar.copy(t[:], g_i[:])
                else:
                    nc.vector.tensor_copy(t[:], g_i[:])
                g_bf[v] = t

            # --- matmuls: psum[64, 1024] over 2 banks of 512 ---
            acc = ps.tile([B, U], f32)
            groups = [(h[v], g_bf[v]) for v in range(1, 16)]
            groups.append((xnz, mneg))
            groups += [(h[v], dmask[128]) for v in (2, 6, 10, 14)]
            groups += [(h[v], dmask[m]) for v in (4, 12) for m in (64, 128)]
            groups += [(h[8], dmask[m]) for m in (32, 64, 96, 128)]
            n_g = len(groups)
            for half in range(2):
                sl = slice(half * 512, (half + 1) * 512)
                for gidx, (lhsT, rhs) in enumerate(groups):
                    nc.tensor.matmul(
                        acc[:, sl], lhsT[:], rhs[:, sl],
                        start=(gidx == 0), stop=(gidx == n_g - 1),
                    )

            # --- epilogue: PSUM -> SBUF fp32, DMA out ---
            o_sb = io.tile([B, U], f32, tag="osb")
            nc.scalar.copy(o_sb[:, 0:512], acc[:, 0:512])
            nc.vector.tensor_copy(o_sb[:, 512:1024], acc[:, 512:1024])
            nc.sync.dma_start(out_d[:], o_sb[:])

    nc.compile()
    return nc


def _get_nc():
    if "nc" not in _NC_CACHE:
        _NC_CACHE["nc"] = _build_nc()
    return _NC_CACHE["nc"]


_LAST_RESULTS = {}


def _kernel_numpy(inputs, bits, kernel, bias):
    # generic (non-8-bit) fallback; mirrors the reference exactly
    x = np.asarray(inputs, np.float64)
    w = np.asarray(kernel, np.float64)
    b = int(bits)
    out = np.zeros((x.shape[0], w.shape[1]), np.float64)
    scale = float(2 ** b)
    for d0 in range(0, w.shape[0], 128):
        d1 = min(d0 + 128, w.shape[0])
        wm = np.sign(w[None, d0:d1, :]) * (
            np.abs(w[None, d0:d1, :]) % scale if b < 31 else np.abs(w[None, d0:d1, :])
        )
        out += np.trunc(x[:, d0:d1, None] * wm / scale).sum(1)
    return np.maximum(out + np.asarray(bias, np.float64)[None, :], 0.0).astype(
        np.float32
    )


def kernel(inputs, bits, kernel, bias):
    if int(bits) != BITS:
        return _kernel_numpy(inputs, bits, kernel, bias)

    from concourse.bass_utils import run_bass_kernel_spmd

    x = np.asarray(inputs)
    w = np.asarray(kernel)
    b = np.asarray(bias, dtype=np.float32)
    assert x.shape == (B, D) and w.shape == (D, U)

    xt = np.ascontiguousarray(x.T.astype(np.int16))  # [D, B]
    wi = np.ascontiguousarray(w.astype(np.int16))    # [D, U]

    in_maps = [
        {
            "xt": np.ascontiguousarray(xt[c * DSH:(c + 1) * DSH]),
            "w": np.ascontiguousarray(wi[c * DSH:(c + 1) * DSH]),
        }
        for c in range(NCORES)
    ]

    nc = _get_nc()
    res = run_bass_kernel_spmd(
        nc, in_maps, core_ids=list(range(NCORES)), trace=TRACE
    )
    _LAST_RESULTS["res"] = res

    total = np.zeros((B, U), dtype=np.float32)
    for r in res.results:
        total += r["out"]
    return np.maximum(total + b[None, :], 0.0).astype(np.float32)


# revision 5
# speedup vs baseline: 1.1820x; 1.1820x over previous
"""Trainium2 Bass kernel for nn_CustomDense (bit-serial quantized dense layer).

Math: the reference's per-element bit-serial shift-add loop computes exactly
    f(x, w) = trunc(x * w / 256)          (bits=8, x in [0,15], w in [-128,127])
so  out = relu(sum_j f(x_ij, w_ju) + bias_u).

Device algorithm (exact, integer-precise):
  one-hot over the 15 nonzero activation values v:
      sum_j f = sum_v (X==v) @ floor(v*W/256) + (trunc - floor) correction.

  G_v is produced in ONE dve op per v via a magic-number trick: the DVE
  computes z = w*(v/256) + (1536 - 511/1024) in fp32 and writes fp16; fp16
  spacing is exactly 1.0 on [1024, 2048), and z is never a tie (4m-511 is
  odd), so round-to-nearest-fp16 gives exactly 1536 + floor(v*w/256).
  The spurious +1536 per product sums to 1536*nnz(x_i), cancelled exactly by
  the Xnz @ (Mneg - 1536) group (values -1536/-1535 are fp16-exact).

  trunc-floor correction: trunc = floor + 1[w<0 and x*|w| % 256 != 0]:
      + Xnz@Mneg - XE1@D128 - XE2@(D64+D128) - XE3@(D32+D64+D96+D128)
  with XE1=H2+H6+H10+H14, XE2=H4+H12, XE3=H8 (sums of existing one-hot
  masks), Dm=1[w=-m] (negated on-chip).

All matmul operands are fp16 (0/1 masks, small ints, 1536+-8: all exact);
PSUM accumulates in fp32 and every partial sum stays < 2^24, so the whole
pipeline is integer-exact.

Sharding: D (contraction, 1024) split across 8 cores, 128 rows each; every
core computes a full [64, 1024] partial in PSUM. Host sums the 8 partials
(exact), adds bias in fp32 and applies relu -- bit-identical to the
reference.
"""

import numpy as np

B, D, U, BITS = 64, 1024, 1024, 8
NCORES = 8
DSH = D // NCORES  # 128 contraction rows per core
MAGIC = 1536.0
OFF = MAGIC - 511.0 / 1024.0

# engine for each G_v pass: "dve" or "act"
G_ENGINE = {v: ("act" if v in () else "dve") for v in range(1, 16)}
TRACE = False

_NC_CACHE = {}


def _build_nc():
    import concourse.bacc as bacc
    import concourse.mybir as mybir
    import concourse.tile as tile

    Alu = mybir.AluOpType
    f16 = mybir.dt.float16
    i16 = mybir.dt.int16
    f32 = mybir.dt.float32

    nc = bacc.Bacc("TRN2", target_bir_lowering=False, debug=False)
    xt_d = nc.dram_tensor("xt", [DSH, B], i16, kind="ExternalInput")
    w_d = nc.dram_tensor("w", [DSH, U], i16, kind="ExternalInput")
    out_d = nc.dram_tensor("out", [B, U], f32, kind="ExternalOutput")

    with tile.TileContext(nc) as tc:
        with (
            tc.tile_pool(name="io", bufs=1) as io,
            tc.tile_pool(name="ps", bufs=1, space="PSUM") as ps,
        ):
            xt_sb = io.tile([DSH, B], i16)
            w_sb = io.tile([DSH, U], i16)
            nc.sync.dma_start(xt_sb[:], xt_d[:])
            nc.sync.dma_start(w_sb[:], w_d[:])

            # --- X-side one-hot masks (fp16 [DSH, B]) ---
            h = {}
            for v in range(1, 16):
                t = io.tile([DSH, B], f16, tag=f"h{v}")
                nc.vector.tensor_scalar(
                    out=t[:], in0=xt_sb[:], scalar1=float(v), scalar2=None,
                    op0=Alu.is_equal,
                )
                h[v] = t
            xnz = io.tile([DSH, B], f16, tag="xnz")
            nc.vector.tensor_scalar(
                out=xnz[:], in0=xt_sb[:], scalar1=1.0, scalar2=None,
                op0=Alu.is_ge,
            )
            # XE sums of one-hot masks (disjoint -> still 0/1)
            xe1 = io.tile([DSH, B], f16, tag="xe1")
            xe1b = io.tile([DSH, B], f16, tag="xe1b")
            nc.vector.tensor_tensor(
                out=xe1b[:], in0=h[2][:], in1=h[6][:], op=Alu.add
            )
            nc.vector.tensor_tensor(
                out=xe1[:], in0=h[10][:], in1=h[14][:], op=Alu.add
            )
            nc.vector.tensor_tensor(
                out=xe1[:], in0=xe1[:], in1=xe1b[:], op=Alu.add
            )
            xe2 = io.tile([DSH, B], f16, tag="xe2")
            nc.vector.tensor_tensor(
                out=xe2[:], in0=h[4][:], in1=h[12][:], op=Alu.add
            )

            # --- W-side masks (fp16 [DSH, U]) ---
            # mneg' = 1[w<0] - 1536   (cancels the +1536 magic in each G_v)
            mneg = io.tile([DSH, U], f16, tag="mneg")
            nc.vector.tensor_scalar(
                out=mneg[:], in0=w_sb[:], scalar1=0.0, scalar2=-MAGIC,
                op0=Alu.is_lt, op1=Alu.add,
            )
            dmask = {}
            for m in (128, 64, 32, 96):
                t = io.tile([DSH, U], f16, tag=f"d{m}")
                nc.vector.tensor_scalar(
                    out=t[:], in0=w_sb[:], scalar1=float(-m), scalar2=-1.0,
                    op0=Alu.is_equal, op1=Alu.mult,
                )
                dmask[m] = t
            # p2n = -(1[w=-64] + 1[w=-128]);  p3n = p2n - 1[w=-32] - 1[w=-96]
            p2n = io.tile([DSH, U], f16, tag="p2n")
            nc.vector.tensor_tensor(
                out=p2n[:], in0=dmask[64][:], in1=dmask[128][:], op=Alu.add
            )
            p3n = io.tile([DSH, U], f16, tag="p3n")
            nc.vector.tensor_tensor(
                out=p3n[:], in0=dmask[32][:], in1=dmask[96][:], op=Alu.add
            )
            nc.vector.tensor_tensor(
                out=p3n[:], in0=p3n[:], in1=p2n[:], op=Alu.add
            )

            # --- G_v = 1536 + floor(v*W/256), fp16, one op per v ---
            g = {}
            for v in range(1, 16):
                t = io.tile([DSH, U], f16, tag=f"g{v}")
                if G_ENGINE[v] == "act":
                    off_sb = io.tile([DSH, 1], f32, tag="offsb")
                    nc.gpsimd.memset(off_sb[:], OFF)
                    nc.scalar.activation(
                        t[:], w_sb[:], mybir.ActivationFunctionType.Identity,
                        bias=off_sb[:], scale=float(v) / 256.0,
                    )
                else:
                    nc.vector.tensor_scalar(
                        out=t[:], in0=w_sb[:], scalar1=float(v) / 256.0,
                        scalar2=OFF, op0=Alu.mult, op1=Alu.add,
                    )
                g[v] = t

            # --- matmul groups: 19 per half (psum [64, 1024] = 2 banks) ---
            acc = ps.tile([B, U], f32)
            groups = [(h[v], g[v]) for v in range(1, 16)]
            groups.append((xnz, mneg))
            groups += [(xe1, dmask[128]), (xe2, p2n), (h[8], p3n)]
            n_g = len(groups)
            for gidx, (lhsT, rhs) in enumerate(groups):
                for half in range(2):
                    sl = slice(half * 512, (half + 1) * 512)
                    nc.tensor.matmul(
                        acc[:, sl], lhsT[:], rhs[:, sl],
                        start=(gidx == 0), stop=(gidx == n_g - 1),
                    )

            # --- epilogue: PSUM -> SBUF fp32, DMA out ---
            o_sb = io.tile([B, U], f32, tag="osb")
            nc.scalar.copy(o_sb[:, 0:512], acc[:, 0:512])
            nc.vector.tensor_copy(o_sb[:, 512:1024], acc[:, 512:1024])
            nc.sync.dma_start(out_d[:], o_sb[:])

    nc.compile()
    return nc


def _get_nc():
    if "nc" not in _NC_CACHE:
        _NC_CACHE["nc"] = _build_nc()
    return _NC_CACHE["nc"]


_LAST_RESULTS = {}


def _kernel_numpy(inputs, bits, kernel, bias):
    # generic (non-8-bit) fallback; mirrors the reference exactly
    x = np.asarray(inputs, np.float64)
    w = np.asarray(kernel, np.float64)
    b = int(bits)
    out = np.zeros((x.shape[0], w.shape[1]), np.float64)
    scale = float(2 ** b)
    for d0 in range(0, w.shape[0], 128):
        d1 = min(d0 + 128, w.shape[0])
        wm = np.sign(w[None, d0:d1, :]) * (
            np.abs(w[None, d0:d1, :]) % scale if b < 31 else np.abs(w[None, d0:d1, :])
        )
        out += np.trunc(x[:, d0:d1, None] * wm / scale).sum(1)
    return np.maximum(out + np.asarray(bias, np.float64)[None, :], 0.0).astype(
        np.float32
    )


def kernel(inputs, bits, kernel, bias):
    if int(bits) != BITS:
        return _kernel_numpy(inputs, bits, kernel, bias)

    from concourse.bass_utils import run_bass_kernel_spmd

    x = np.asarray(inputs)
    w = np.asarray(kernel)
    b = np.asarray(bias, dtype=np.float32)
    assert x.shape == (B, D) and w.shape == (D, U)

    xt = np.ascontiguousarray(x.T.astype(np.int16))  # [D, B]
    wi = np.ascontiguousarray(w.astype(np.int16))    # [D, U]

    in_maps = [
        {
            "xt": np.ascontiguousarray(xt[c * DSH:(c + 1) * DSH]),
            "w": np.ascontiguousarray(wi[c * DSH:(c + 1) * DSH]),
        }
        for c in range(NCORES)
    ]

    nc = _get_nc()
    res = run_bass_kernel_spmd(
        nc, in_maps, core_ids=list(range(NCORES)), trace=TRACE
    )
    _LAST_RESULTS["res"] = res

    total = np.zeros((B, U), dtype=np.float32)
    for r in res.results:
        total += r["out"]
    return np.maximum(total + b[None, :], 0.0).astype(np.float32)
